# revision 8
# baseline (speedup 1.0000x reference)
"""MoE layer (8 experts, top-2 routing + shared expert) on 8 Trainium2 cores.

Strategy (expert parallelism per the sharding hint):
  - Host computes the router (logits -> softmax -> top-2 -> combine weights)
    and *dispatches*: core e receives the tokens routed to expert e (gathered,
    transposed to [D, C] layout, bf16) plus a 1/8 data-parallel slice of all
    tokens for the shared expert.
  - Each core runs one Bass/Tile kernel computing, for its token set,
      y = (silu(x @ Wg.T) * (x @ Wu.T)) @ Wd.T   (scaled by combine weight)
    for its expert's weights, then the same with the shared-expert weights.
    All matmuls are bf16 with fp32 PSUM accumulation.
  - Host *combines*: scatter-adds the per-expert outputs and the shared
    outputs back into the full [N, D] result.

Device layout per core (SPMD, one NEFF):
  xt  [D, TT]  bf16   tokens on the free dim, D on partitions (16 k-tiles)
  wg,wu [D, H] bf16   expert-then-shared weight loads (H on free dim)
  wd  [H, D]   bf16
  cw  [TT]     f32    per-token combine weight (1.0 for the shared slice)
  y   [TT, D]  f32    output, tokens on partitions at write time

Pipeline per 512-token chunk: 2*11*16 matmuls produce g,u in PSUM per
128-row H tile; ScalarE applies Silu, VectorE multiplies into a bf16 act
tile [H, chunk]; 4x4x11 matmuls then contract act.T @ WdT into [128 tokens,
512 D] PSUM tiles, which VectorE scales by cw and DMAs out.
"""

import os

import numpy as np
import ml_dtypes

import concourse.bass as bass
import concourse.mybir as mybir
import concourse.tile as tile
from concourse import bacc
from concourse.bass import ds
from concourse.bass_utils import run_bass_kernel_spmd

P = 128
D = 2048
H = 1408
E = 8
TOP_K = 2
KD = D // P   # 16
KH = H // P   # 11
BF16 = mybir.dt.bfloat16
F32 = mybir.dt.float32


def _chunks(count, base):
    """Split `count` tokens (multiple of 128) into chunks of 512 then 128."""
    out = []
    pos = 0
    while count - pos >= 512:
        out.append((base + pos, 512))
        pos += 512
    while count - pos >= P:
        out.append((base + pos, P))
        pos += P
    assert pos == count
    return out


def build_kernel(C, S, repeat=1, xb=2, ab=2, ob=2, pgu=3, pyb=2):
    """Build the SPMD Bass module for C expert tokens + S shared tokens."""
    TT = C + S
    assert C % P == 0 and S % P == 0

    nc = bacc.Bacc(
        "TRN2",
        target_bir_lowering=False,
        debug=False,
        enable_asserts=False,
        num_devices=8,
    )

    xt = nc.dram_tensor("xt", [D, TT], BF16, kind="ExternalInput").ap()
    wts = {}
    for pref in ("e", "s"):
        wts[pref] = (
            nc.dram_tensor(f"wg_{pref}", [D, H], BF16, kind="ExternalInput").ap(),
            nc.dram_tensor(f"wu_{pref}", [D, H], BF16, kind="ExternalInput").ap(),
            nc.dram_tensor(f"wd_{pref}", [H, D], BF16, kind="ExternalInput").ap(),
        )
    cw = nc.dram_tensor("cw", [P, TT // P], F32, kind="ExternalInput").ap()
    y = nc.dram_tensor("y", [TT, D], F32, kind="ExternalOutput").ap()

    xt_r = xt.rearrange("(ko p) t -> p ko t", p=P)     # [128, 16, TT]
    y_r = y.rearrange("(g p) d -> p g d", p=P)         # [128, TT/128, 2048]
    cw_r = cw  # already [128, TT/128] host-transposed

    phases = [("e", 0, C), ("s", C, S)]

    with tile.TileContext(nc) as tc:
        with (
            tc.tile_pool(name="wgp", bufs=1) as wgp,
            tc.tile_pool(name="wup", bufs=1) as wup,
            tc.tile_pool(name="wdp", bufs=1) as wdp,
            tc.tile_pool(name="xp", bufs=xb) as xp,
            tc.tile_pool(name="ap", bufs=ab) as apool,
            tc.tile_pool(name="op", bufs=ob) as opool,
            tc.tile_pool(name="cp", bufs=1) as cpool,
            tc.tile_pool(name="psgu", bufs=pgu, space="PSUM") as psgu,
            tc.tile_pool(name="psy", bufs=pyb, space="PSUM") as psy,
        ):
            cw_sb = cpool.tile([P, TT // P], F32)
            nc.sync.dma_start(cw_sb[:], cw_r)

            for pref, base, count in phases * repeat:
                if count == 0:
                    continue
                wg_d, wu_d, wd_d = wts[pref]
                wg_sb = wgp.tile([P, KD, H], BF16, tag="wg")
                wu_sb = wup.tile([P, KD, H], BF16, tag="wu")
                wg_rr = wg_d.rearrange("(ko p) h -> p ko h", p=P)
                wu_rr = wu_d.rearrange("(ko p) h -> p ko h", p=P)
                chunk_list = _chunks(count, base)
                # interleave chunk-0's x tiles with the weight k-tiles so the
                # first matmuls' operands land first (per-queue DMA bandwidth
                # is the prologue limiter)
                start0, w0 = chunk_list[0]
                x0_sb = xp.tile([P, KD, 512], BF16, tag="x", name="x0_sb")[:, :, :w0]
                for k in range(KD):
                    nc.sync.dma_start(x0_sb[:, k, :], xt_r[:, k, ds(start0, w0)])
                    nc.sync.dma_start(wg_sb[:, k, :], wg_rr[:, k, :])
                    nc.sync.dma_start(wu_sb[:, k, :], wu_rr[:, k, :])
                wd_sb = wdp.tile([P, KH, D], BF16, tag="wd")
                wd_rr = wd_d.rearrange("(ho p) d -> p ho d", p=P)
                for h in range(KH):
                    nc.sync.dma_start(wd_sb[:, h, :], wd_rr[:, h, :])

                for ci, (start, w) in enumerate(chunk_list):
                    if ci == 0:
                        x_sb = x0_sb
                    else:
                        x_sb = xp.tile([P, KD, 512], BF16, tag="x", name="x_sb")[:, :, :w]
                        for k in range(KD):
                            nc.sync.dma_start(x_sb[:, k, :], xt_r[:, k, ds(start, w)])

                    aT = apool.tile([P, KH, 512], BF16, tag="a", name="aT")[:, :, :w]
                    for h in range(KH):
                        pg = psgu.tile([P, 512], F32, tag="psg", name="pg")[:, :w]
                        pu = psgu.tile([P, 512], F32, tag="psu", name="pu")[:, :w]
                        for k in range(KD):
                            nc.tensor.matmul(
                                pg,
                                wg_sb[:, k, h * P : (h + 1) * P],
                                x_sb[:, k, :],
                                start=(k == 0),
                                stop=(k == KD - 1),
                            )
                        for k in range(KD):
                            nc.tensor.matmul(
                                pu,
                                wu_sb[:, k, h * P : (h + 1) * P],
                                x_sb[:, k, :],
                                start=(k == 0),
                                stop=(k == KD - 1),
                            )
                        nc.scalar.activation(
                            aT[:, h, :], pg, mybir.ActivationFunctionType.Silu
                        )
                        nc.vector.tensor_tensor(
                            aT[:, h, :], aT[:, h, :], pu, mybir.AluOpType.mult
                        )

                    for g in range(w // P):
                        gg = (start + g * P) // P
                        out_sb = opool.tile([P, 4, 512], F32, tag="o", name="out_sb")
                        for d4 in range(4):
                            py = psy.tile([P, 512], F32, tag="psy", name="py")
                            for h in range(KH):
                                nc.tensor.matmul(
                                    py,
                                    aT[:, h, g * P : (g + 1) * P],
                                    wd_sb[:, h, d4 * 512 : (d4 + 1) * 512],
                                    start=(h == 0),
                                    stop=(h == KH - 1),
                                )
                            nc.vector.tensor_scalar_mul(
                                out_sb[:, d4, :], py, cw_sb[:, gg : gg + 1]
                            )
                        nc.sync.dma_start(y_r[:, gg, :], out_sb[:])

    nc.compile()
    return nc


def _route(x_flat, gate_w, expert_bias):
    """Replicate the reference router in numpy (fp32)."""
    N = x_flat.shape[0]
    logits = x_flat @ gate_w.T                       # [N, E]
    m = logits.max(-1, keepdims=True)
    p = np.exp(logits - m)
    p /= p.sum(-1, keepdims=True)
    biased = logits + expert_bias
    rows = np.arange(N)
    i1 = biased.argmax(-1)
    b2 = biased.copy()
    b2[rows, i1] = -np.inf
    i2 = b2.argmax(-1)
    w1 = p[rows, i1]
    w2 = p[rows, i2]
    s = w1 + w2
    return i1, i2, w1 / s, w2 / s


def _prepare(inputs):
    x = np.asarray(inputs["x"], dtype=np.float32)
    B, S_, D_ = x.shape
    assert D_ == D
    x_flat = x.reshape(-1, D)
    N = x_flat.shape[0]
    S0 = N // 8

    i1, i2, w1, w2 = _route(
        x_flat,
        np.asarray(inputs["gate_w"], dtype=np.float32),
        np.asarray(inputs["expert_bias"], dtype=np.float32),
    )

    idx_lists = []
    w_lists = []
    for e in range(E):
        m1 = i1 == e
        m2 = i2 == e
        idx = np.nonzero(m1 | m2)[0]
        w = np.where(m1[idx], w1[idx], w2[idx]).astype(np.float32)
        idx_lists.append(idx)
        w_lists.append(w)

    maxc = max(len(ix) for ix in idx_lists)
    C = ((maxc + P - 1) // P) * P
    TT = C + S0

    bf = ml_dtypes.bfloat16
    Wg = np.asarray(inputs["Wg"], dtype=np.float32)
    Wu = np.asarray(inputs["Wu"], dtype=np.float32)
    Wd = np.asarray(inputs["Wd"], dtype=np.float32)
    wsg = np.ascontiguousarray(np.asarray(inputs["Ws_g"], np.float32).T).astype(bf)
    wsu = np.ascontiguousarray(np.asarray(inputs["Ws_u"], np.float32).T).astype(bf)
    wsd = np.ascontiguousarray(np.asarray(inputs["Ws_d"], np.float32).T).astype(bf)

    in_maps = []
    idx_pad = np.empty((E, C), dtype=np.int64)
    for e in range(E):
        idx = idx_lists[e]
        pad = np.full(C - len(idx), N, dtype=np.int64)  # N -> dummy row
        idx_pad[e] = np.concatenate([idx, pad])
        gather_idx = np.concatenate([idx, np.zeros(C - len(idx), np.int64)])

        xt = np.empty((D, TT), dtype=bf)
        xt[:, :C] = x_flat[gather_idx].T
        xt[:, C:] = x_flat[e * S0 : (e + 1) * S0].T

        cwv = np.ones(TT, dtype=np.float32)
        cwv[: len(idx)] = w_lists[e]
        cwv[len(idx) : C] = 0.0
        cwv = np.ascontiguousarray(cwv.reshape(TT // P, P).T)

        in_maps.append(
            {
                "xt": xt,
                "wg_e": np.ascontiguousarray(Wg[e].T).astype(bf),
                "wu_e": np.ascontiguousarray(Wu[e].T).astype(bf),
                "wd_e": np.ascontiguousarray(Wd[e].T).astype(bf),
                "wg_s": wsg,
                "wu_s": wsu,
                "wd_s": wsd,
                "cw": cwv,
            }
        )
    return x, in_maps, idx_pad, C, S0, N


def _combine(x_shape, results, idx_pad, C, S0, N):
    acc = np.zeros((N + 1, D), dtype=np.float32)
    for e in range(E):
        ye = results[e]["y"]
        acc[idx_pad[e]] += ye[:C]
        acc[e * S0 : (e + 1) * S0] += ye[C:]
    return acc[:N].reshape(x_shape)


def kernel(**inputs) -> np.ndarray:
    x, in_maps, idx_pad, C, S0, N = _prepare(inputs)
    nc = build_kernel(C, S0)
    res = run_bass_kernel_spmd(nc, in_maps, core_ids=list(range(8)))
    return _combine(x.shape, [res.results[e] for e in range(E)], idx_pad, C, S0, N)


# revision 10
# speedup vs baseline: 3.5345x; 3.5345x over previous
"""MoE layer (8 experts, top-2 routing + shared expert) on 8 Trainium2 cores.

Strategy (expert parallelism per the sharding hint):
  - Host computes the router (logits -> softmax -> top-2 -> combine weights)
    and *dispatches*: core e receives the tokens routed to expert e (gathered,
    transposed to [D, C] layout, bf16) plus a 1/8 data-parallel slice of all
    tokens for the shared expert.
  - Each core runs one Bass/Tile kernel computing, for its token set,
      y = (silu(x @ Wg.T) * (x @ Wu.T)) @ Wd.T   (scaled by combine weight)
    for its expert's weights, then the same with the shared-expert weights.
    All matmuls are bf16 with fp32 PSUM accumulation.
  - Host *combines*: scatter-adds the per-expert outputs and the shared
    outputs back into the full [N, D] result.

Device layout per core (SPMD, one NEFF):
  xt  [D, TT]  bf16   tokens on the free dim, D on partitions (16 k-tiles)
  wg,wu [D, H] bf16   expert-then-shared weight loads (H on free dim)
  wd  [H, D]   bf16
  cw  [128, TT/128] f32  per-token combine weight, pre-grouped on host so
                         the DMA is contiguous (1.0 for the shared slice)
  y   [TT, D]  f32    output, tokens on partitions at write time

Pipeline per 512-token chunk: 2*11*16 matmuls produce g,u in PSUM per
128-row H tile; ScalarE applies Silu, VectorE multiplies into a bf16 act
tile [H, chunk]; 4x4x11 matmuls then contract act.T @ WdT into [128 tokens,
512 D] PSUM tiles, which VectorE scales by cw and DMAs out.
"""

import os

import numpy as np
import ml_dtypes

import concourse.bass as bass
import concourse.mybir as mybir
import concourse.tile as tile
from concourse import bacc
from concourse.bass import ds
from concourse.bass_utils import run_bass_kernel_spmd

P = 128
D = 2048
H = 1408
E = 8
TOP_K = 2
KD = D // P   # 16
KH = H // P   # 11
BF16 = mybir.dt.bfloat16
F32 = mybir.dt.float32


def _chunks(count, base):
    """Split `count` tokens (multiple of 128) into chunks of 512 then 128."""
    out = []
    pos = 0
    while count - pos >= 512:
        out.append((base + pos, 512))
        pos += 512
    while count - pos >= P:
        out.append((base + pos, P))
        pos += P
    assert pos == count
    return out


def build_kernel(C, S, repeat=1, xb=2, ab=2, ob=2, pgu=3, pyb=2, wd_late=False, ysplit=True):
    """Build the SPMD Bass module for C expert tokens + S shared tokens."""
    TT = C + S
    assert C % P == 0 and S % P == 0

    nc = bacc.Bacc(
        "TRN2",
        target_bir_lowering=False,
        debug=False,
        enable_asserts=False,
        num_devices=8,
    )

    xt = nc.dram_tensor("xt", [D, TT], BF16, kind="ExternalInput").ap()
    wts = {}
    for pref in ("e", "s"):
        wts[pref] = (
            nc.dram_tensor(f"wg_{pref}", [D, H], BF16, kind="ExternalInput").ap(),
            nc.dram_tensor(f"wu_{pref}", [D, H], BF16, kind="ExternalInput").ap(),
            nc.dram_tensor(f"wd_{pref}", [H, D], BF16, kind="ExternalInput").ap(),
        )
    cw = nc.dram_tensor("cw", [P, TT // P], F32, kind="ExternalInput").ap()
    y = nc.dram_tensor("y", [TT, D], F32, kind="ExternalOutput").ap()

    xt_r = xt.rearrange("(ko p) t -> p ko t", p=P)     # [128, 16, TT]
    y_r = y.rearrange("(g p) d -> p g d", p=P)         # [128, TT/128, 2048]
    cw_r = cw  # already [128, TT/128] host-transposed

    phases = [("e", 0, C), ("s", C, S)]

    with tile.TileContext(nc) as tc:
        with (
            tc.tile_pool(name="wgp", bufs=1) as wgp,
            tc.tile_pool(name="wup", bufs=1) as wup,
            tc.tile_pool(name="wdp", bufs=1) as wdp,
            tc.tile_pool(name="xp", bufs=xb) as xp,
            tc.tile_pool(name="ap", bufs=ab) as apool,
            tc.tile_pool(name="op", bufs=ob) as opool,
            tc.tile_pool(name="cp", bufs=1) as cpool,
            tc.tile_pool(name="psgu", bufs=pgu, space="PSUM") as psgu,
            tc.tile_pool(name="psy", bufs=pyb, space="PSUM") as psy,
        ):
            cw_sb = cpool.tile([P, TT // P], F32)
            nc.sync.dma_start(cw_sb[:], cw_r)

            for pref, base, count in phases * repeat:
                if count == 0:
                    continue
                wg_d, wu_d, wd_d = wts[pref]
                wg_sb = wgp.tile([P, KD, H], BF16, tag="wg")
                wu_sb = wup.tile([P, KD, H], BF16, tag="wu")
                wg_rr = wg_d.rearrange("(ko p) h -> p ko h", p=P)
                wu_rr = wu_d.rearrange("(ko p) h -> p ko h", p=P)
                chunk_list = _chunks(count, base)
                # interleave chunk-0's x tiles with the weight k-tiles so the
                # first matmuls' operands land first (per-queue DMA bandwidth
                # is the prologue limiter)
                start0, w0 = chunk_list[0]
                x0_sb = xp.tile([P, KD, 512], BF16, tag="x", name="x0_sb")[:, :, :w0]
                for k in range(KD):
                    nc.sync.dma_start(x0_sb[:, k, :], xt_r[:, k, ds(start0, w0)])
                    nc.sync.dma_start(wg_sb[:, k, :], wg_rr[:, k, :])
                    nc.sync.dma_start(wu_sb[:, k, :], wu_rr[:, k, :])
                wd_sb = wdp.tile([P, KH, D], BF16, tag="wd")
                wd_rr = wd_d.rearrange("(ho p) d -> p ho d", p=P)
                if not wd_late:
                    for h in range(KH):
                        nc.sync.dma_start(wd_sb[:, h, :], wd_rr[:, h, :])

                for ci, (start, w) in enumerate(chunk_list):
                    if wd_late and ci == 1:
                        for h in range(KH):
                            nc.sync.dma_start(wd_sb[:, h, :], wd_rr[:, h, :])
                    if ci == 0:
                        x_sb = x0_sb
                    else:
                        x_sb = xp.tile([P, KD, 512], BF16, tag="x", name="x_sb")[:, :, :w]
                        for k in range(KD):
                            nc.sync.dma_start(x_sb[:, k, :], xt_r[:, k, ds(start, w)])

                    aT = apool.tile([P, KH, 512], BF16, tag="a", name="aT")[:, :, :w]
                    for h in range(KH):
                        pg = psgu.tile([P, 512], F32, tag="psg", name="pg")[:, :w]
                        pu = psgu.tile([P, 512], F32, tag="psu", name="pu")[:, :w]
                        for k in range(KD):
                            nc.tensor.matmul(
                                pg,
                                wg_sb[:, k, h * P : (h + 1) * P],
                                x_sb[:, k, :],
                                start=(k == 0),
                                stop=(k == KD - 1),
                            )
                        for k in range(KD):
                            nc.tensor.matmul(
                                pu,
                                wu_sb[:, k, h * P : (h + 1) * P],
                                x_sb[:, k, :],
                                start=(k == 0),
                                stop=(k == KD - 1),
                            )
                        nc.scalar.activation(
                            aT[:, h, :], pg, mybir.ActivationFunctionType.Silu
                        )
                        nc.vector.tensor_tensor(
                            aT[:, h, :], aT[:, h, :], pu, mybir.AluOpType.mult
                        )

                    for g in range(w // P):
                        gg = (start + g * P) // P
                        out_sb = opool.tile([P, 4, 512], F32, tag="o", name="out_sb")
                        for d4 in range(4):
                            py = psy.tile([P, 512], F32, tag="psy", name="py")
                            for h in range(KH):
                                nc.tensor.matmul(
                                    py,
                                    aT[:, h, g * P : (g + 1) * P],
                                    wd_sb[:, h, d4 * 512 : (d4 + 1) * 512],
                                    start=(h == 0),
                                    stop=(h == KH - 1),
                                )
                            nc.vector.tensor_scalar_mul(
                                out_sb[:, d4, :], py, cw_sb[:, gg : gg + 1]
                            )
                        if ysplit:
                            for d4 in range(4):
                                nc.sync.dma_start(
                                    y_r[:, gg, d4 * 512 : (d4 + 1) * 512],
                                    out_sb[:, d4, :],
                                )
                        else:
                            nc.sync.dma_start(y_r[:, gg, :], out_sb[:])

    nc.compile()
    return nc


def _route(x_flat, gate_w, expert_bias):
    """Replicate the reference router in numpy (fp32)."""
    N = x_flat.shape[0]
    logits = x_flat @ gate_w.T                       # [N, E]
    m = logits.max(-1, keepdims=True)
    p = np.exp(logits - m)
    p /= p.sum(-1, keepdims=True)
    biased = logits + expert_bias
    rows = np.arange(N)
    i1 = biased.argmax(-1)
    b2 = biased.copy()
    b2[rows, i1] = -np.inf
    i2 = b2.argmax(-1)
    w1 = p[rows, i1]
    w2 = p[rows, i2]
    s = w1 + w2
    return i1, i2, w1 / s, w2 / s


def _prepare(inputs):
    x = np.asarray(inputs["x"], dtype=np.float32)
    B, S_, D_ = x.shape
    assert D_ == D
    x_flat = x.reshape(-1, D)
    N = x_flat.shape[0]
    S0 = N // 8

    i1, i2, w1, w2 = _route(
        x_flat,
        np.asarray(inputs["gate_w"], dtype=np.float32),
        np.asarray(inputs["expert_bias"], dtype=np.float32),
    )

    idx_lists = []
    w_lists = []
    for e in range(E):
        m1 = i1 == e
        m2 = i2 == e
        idx = np.nonzero(m1 | m2)[0]
        w = np.where(m1[idx], w1[idx], w2[idx]).astype(np.float32)
        idx_lists.append(idx)
        w_lists.append(w)

    maxc = max(len(ix) for ix in idx_lists)
    C = ((maxc + P - 1) // P) * P
    TT = C + S0

    bf = ml_dtypes.bfloat16
    Wg = np.asarray(inputs["Wg"], dtype=np.float32)
    Wu = np.asarray(inputs["Wu"], dtype=np.float32)
    Wd = np.asarray(inputs["Wd"], dtype=np.float32)
    wsg = np.ascontiguousarray(np.asarray(inputs["Ws_g"], np.float32).T).astype(bf)
    wsu = np.ascontiguousarray(np.asarray(inputs["Ws_u"], np.float32).T).astype(bf)
    wsd = np.ascontiguousarray(np.asarray(inputs["Ws_d"], np.float32).T).astype(bf)

    in_maps = []
    idx_pad = np.empty((E, C), dtype=np.int64)
    for e in range(E):
        idx = idx_lists[e]
        pad = np.full(C - len(idx), N, dtype=np.int64)  # N -> dummy row
        idx_pad[e] = np.concatenate([idx, pad])
        gather_idx = np.concatenate([idx, np.zeros(C - len(idx), np.int64)])

        xt = np.empty((D, TT), dtype=bf)
        xt[:, :C] = x_flat[gather_idx].T
        xt[:, C:] = x_flat[e * S0 : (e + 1) * S0].T

        cwv = np.ones(TT, dtype=np.float32)
        cwv[: len(idx)] = w_lists[e]
        cwv[len(idx) : C] = 0.0
        cwv = np.ascontiguousarray(cwv.reshape(TT // P, P).T)

        in_maps.append(
            {
                "xt": xt,
                "wg_e": np.ascontiguousarray(Wg[e].T).astype(bf),
                "wu_e": np.ascontiguousarray(Wu[e].T).astype(bf),
                "wd_e": np.ascontiguousarray(Wd[e].T).astype(bf),
                "wg_s": wsg,
                "wu_s": wsu,
                "wd_s": wsd,
                "cw": cwv,
            }
        )
    return x, in_maps, idx_pad, C, S0, N


def _combine(x_shape, results, idx_pad, C, S0, N):
    acc = np.zeros((N + 1, D), dtype=np.float32)
    for e in range(E):
        ye = results[e]["y"]
        acc[idx_pad[e]] += ye[:C]
        acc[e * S0 : (e + 1) * S0] += ye[C:]
    return acc[:N].reshape(x_shape)


def kernel(**inputs) -> np.ndarray:
    x, in_maps, idx_pad, C, S0, N = _prepare(inputs)
    nc = build_kernel(C, S0)
    res = run_bass_kernel_spmd(nc, in_maps, core_ids=list(range(8)))
    return _combine(x.shape, [res.results[e] for e in range(E)], idx_pad, C, S0, N)


# revision 11
# speedup vs baseline: 3.7865x; 1.0713x over previous
"""MoE layer (8 experts, top-2 routing + shared expert) on 8 Trainium2 cores.

Strategy (expert parallelism per the sharding hint):
  - Host computes the router (logits -> softmax -> top-2 -> combine weights)
    and *dispatches*: core e receives the tokens routed to expert e (gathered,
    transposed to [D, C] layout, fp16) plus a 1/8 data-parallel slice of all
    tokens for the shared expert.
  - Each core runs one Bass/Tile kernel computing, for its token set,
      y = (silu(x @ Wg.T) * (x @ Wu.T)) @ Wd.T   (scaled by combine weight)
    for its expert's weights, then the same with the shared-expert weights.
    All matmuls are fp16 with fp32 PSUM accumulation.
  - Host *combines*: scatter-adds the per-expert outputs and the shared
    outputs back into the full [N, D] result.

Device layout per core (SPMD, one NEFF):
  xt  [D, TT]  fp16   tokens on the free dim, D on partitions (16 k-tiles)
  wg,wu [D, H] fp16   expert-then-shared weight loads (H on free dim)
  wd  [H, D]   fp16
  cw  [128, TT/128] f32  per-token combine weight, pre-grouped on host so
                         the DMA is contiguous (1.0 for the shared slice)
  y   [TT, D]  f32    output, tokens on partitions at write time

Pipeline per 512-token chunk: 2*11*16 matmuls produce g,u in PSUM per
128-row H tile; ScalarE applies Silu, VectorE multiplies into an fp16 act
tile [H, chunk]; 4x4x11 matmuls then contract act.T @ WdT into [128 tokens,
512 D] PSUM tiles, which VectorE scales by cw and DMAs out.
"""

import os

import numpy as np
import ml_dtypes

import concourse.bass as bass
import concourse.mybir as mybir
import concourse.tile as tile
from concourse import bacc
from concourse.bass import ds
from concourse.bass_utils import run_bass_kernel_spmd

P = 128
D = 2048
H = 1408
E = 8
TOP_K = 2
KD = D // P   # 16
KH = H // P   # 11
DT16 = mybir.dt.float16  # fp16: same PE rate as bf16, 8x the mantissa precision
F32 = mybir.dt.float32


def _chunks(count, base):
    """Split `count` tokens (multiple of 128) into chunks of 512 then 128."""
    out = []
    pos = 0
    while count - pos >= 512:
        out.append((base + pos, 512))
        pos += 512
    while count - pos >= P:
        out.append((base + pos, P))
        pos += P
    assert pos == count
    return out


def build_kernel(C, S, repeat=1, xb=2, ab=2, ob=2, pgu=3, pyb=2, wd_late=False, ysplit=True):
    """Build the SPMD Bass module for C expert tokens + S shared tokens."""
    TT = C + S
    assert C % P == 0 and S % P == 0

    nc = bacc.Bacc(
        "TRN2",
        target_bir_lowering=False,
        debug=False,
        enable_asserts=False,
        num_devices=8,
    )

    xt = nc.dram_tensor("xt", [D, TT], DT16, kind="ExternalInput").ap()
    wts = {}
    for pref in ("e", "s"):
        wts[pref] = (
            nc.dram_tensor(f"wg_{pref}", [D, H], DT16, kind="ExternalInput").ap(),
            nc.dram_tensor(f"wu_{pref}", [D, H], DT16, kind="ExternalInput").ap(),
            nc.dram_tensor(f"wd_{pref}", [H, D], DT16, kind="ExternalInput").ap(),
        )
    cw = nc.dram_tensor("cw", [P, TT // P], F32, kind="ExternalInput").ap()
    y = nc.dram_tensor("y", [TT, D], F32, kind="ExternalOutput").ap()

    xt_r = xt.rearrange("(ko p) t -> p ko t", p=P)     # [128, 16, TT]
    y_r = y.rearrange("(g p) d -> p g d", p=P)         # [128, TT/128, 2048]
    cw_r = cw  # already [128, TT/128] host-transposed

    phases = [("e", 0, C), ("s", C, S)]

    with tile.TileContext(nc) as tc:
        with (
            tc.tile_pool(name="wgp", bufs=1) as wgp,
            tc.tile_pool(name="wup", bufs=1) as wup,
            tc.tile_pool(name="wdp", bufs=1) as wdp,
            tc.tile_pool(name="xp", bufs=xb) as xp,
            tc.tile_pool(name="ap", bufs=ab) as apool,
            tc.tile_pool(name="op", bufs=ob) as opool,
            tc.tile_pool(name="cp", bufs=1) as cpool,
            tc.tile_pool(name="psgu", bufs=pgu, space="PSUM") as psgu,
            tc.tile_pool(name="psy", bufs=pyb, space="PSUM") as psy,
        ):
            cw_sb = cpool.tile([P, TT // P], F32)
            nc.sync.dma_start(cw_sb[:], cw_r)

            for pref, base, count in phases * repeat:
                if count == 0:
                    continue
                wg_d, wu_d, wd_d = wts[pref]
                wg_sb = wgp.tile([P, KD, H], DT16, tag="wg")
                wu_sb = wup.tile([P, KD, H], DT16, tag="wu")
                wg_rr = wg_d.rearrange("(ko p) h -> p ko h", p=P)
                wu_rr = wu_d.rearrange("(ko p) h -> p ko h", p=P)
                chunk_list = _chunks(count, base)
                # interleave chunk-0's x tiles with the weight k-tiles so the
                # first matmuls' operands land first (per-queue DMA bandwidth
                # is the prologue limiter)
                start0, w0 = chunk_list[0]
                x0_sb = xp.tile([P, KD, 512], DT16, tag="x", name="x0_sb")[:, :, :w0]
                for k in range(KD):
                    nc.sync.dma_start(x0_sb[:, k, :], xt_r[:, k, ds(start0, w0)])
                    nc.sync.dma_start(wg_sb[:, k, :], wg_rr[:, k, :])
                    nc.sync.dma_start(wu_sb[:, k, :], wu_rr[:, k, :])
                wd_sb = wdp.tile([P, KH, D], DT16, tag="wd")
                wd_rr = wd_d.rearrange("(ho p) d -> p ho d", p=P)
                if not wd_late:
                    for h in range(KH):
                        nc.sync.dma_start(wd_sb[:, h, :], wd_rr[:, h, :])

                for ci, (start, w) in enumerate(chunk_list):
                    if wd_late and ci == 1:
                        for h in range(KH):
                            nc.sync.dma_start(wd_sb[:, h, :], wd_rr[:, h, :])
                    if ci == 0:
                        x_sb = x0_sb
                    else:
                        x_sb = xp.tile([P, KD, 512], DT16, tag="x", name="x_sb")[:, :, :w]
                        for k in range(KD):
                            nc.sync.dma_start(x_sb[:, k, :], xt_r[:, k, ds(start, w)])

                    aT = apool.tile([P, KH, 512], DT16, tag="a", name="aT")[:, :, :w]
                    for h in range(KH):
                        pg = psgu.tile([P, 512], F32, tag="psg", name="pg")[:, :w]
                        pu = psgu.tile([P, 512], F32, tag="psu", name="pu")[:, :w]
                        for k in range(KD):
                            nc.tensor.matmul(
                                pg,
                                wg_sb[:, k, h * P : (h + 1) * P],
                                x_sb[:, k, :],
                                start=(k == 0),
                                stop=(k == KD - 1),
                            )
                        for k in range(KD):
                            nc.tensor.matmul(
                                pu,
                                wu_sb[:, k, h * P : (h + 1) * P],
                                x_sb[:, k, :],
                                start=(k == 0),
                                stop=(k == KD - 1),
                            )
                        nc.scalar.activation(
                            aT[:, h, :], pg, mybir.ActivationFunctionType.Silu
                        )
                        nc.vector.tensor_tensor(
                            aT[:, h, :], aT[:, h, :], pu, mybir.AluOpType.mult
                        )

                    for g in range(w // P):
                        gg = (start + g * P) // P
                        out_sb = opool.tile([P, 4, 512], F32, tag="o", name="out_sb")
                        for d4 in range(4):
                            py = psy.tile([P, 512], F32, tag="psy", name="py")
                            for h in range(KH):
                                nc.tensor.matmul(
                                    py,
                                    aT[:, h, g * P : (g + 1) * P],
                                    wd_sb[:, h, d4 * 512 : (d4 + 1) * 512],
                                    start=(h == 0),
                                    stop=(h == KH - 1),
                                )
                            nc.vector.tensor_scalar_mul(
                                out_sb[:, d4, :], py, cw_sb[:, gg : gg + 1]
                            )
                        if ysplit:
                            for d4 in range(4):
                                nc.sync.dma_start(
                                    y_r[:, gg, d4 * 512 : (d4 + 1) * 512],
                                    out_sb[:, d4, :],
                                )
                        else:
                            nc.sync.dma_start(y_r[:, gg, :], out_sb[:])

    nc.compile()
    return nc


def _route(x_flat, gate_w, expert_bias):
    """Replicate the reference router in numpy (fp32)."""
    N = x_flat.shape[0]
    logits = x_flat @ gate_w.T                       # [N, E]
    m = logits.max(-1, keepdims=True)
    p = np.exp(logits - m)
    p /= p.sum(-1, keepdims=True)
    biased = logits + expert_bias
    rows = np.arange(N)
    i1 = biased.argmax(-1)
    b2 = biased.copy()
    b2[rows, i1] = -np.inf
    i2 = b2.argmax(-1)
    w1 = p[rows, i1]
    w2 = p[rows, i2]
    s = w1 + w2
    return i1, i2, w1 / s, w2 / s


def _prepare(inputs):
    x = np.asarray(inputs["x"], dtype=np.float32)
    B, S_, D_ = x.shape
    assert D_ == D
    x_flat = x.reshape(-1, D)
    N = x_flat.shape[0]
    S0 = N // 8

    i1, i2, w1, w2 = _route(
        x_flat,
        np.asarray(inputs["gate_w"], dtype=np.float32),
        np.asarray(inputs["expert_bias"], dtype=np.float32),
    )

    idx_lists = []
    w_lists = []
    for e in range(E):
        m1 = i1 == e
        m2 = i2 == e
        idx = np.nonzero(m1 | m2)[0]
        w = np.where(m1[idx], w1[idx], w2[idx]).astype(np.float32)
        idx_lists.append(idx)
        w_lists.append(w)

    maxc = max(len(ix) for ix in idx_lists)
    C = ((maxc + P - 1) // P) * P
    TT = C + S0

    bf = np.float16
    Wg = np.asarray(inputs["Wg"], dtype=np.float32)
    Wu = np.asarray(inputs["Wu"], dtype=np.float32)
    Wd = np.asarray(inputs["Wd"], dtype=np.float32)
    wsg = np.ascontiguousarray(np.asarray(inputs["Ws_g"], np.float32).T).astype(bf)
    wsu = np.ascontiguousarray(np.asarray(inputs["Ws_u"], np.float32).T).astype(bf)
    wsd = np.ascontiguousarray(np.asarray(inputs["Ws_d"], np.float32).T).astype(bf)

    in_maps = []
    idx_pad = np.empty((E, C), dtype=np.int64)
    for e in range(E):
        idx = idx_lists[e]
        pad = np.full(C - len(idx), N, dtype=np.int64)  # N -> dummy row
        idx_pad[e] = np.concatenate([idx, pad])
        gather_idx = np.concatenate([idx, np.zeros(C - len(idx), np.int64)])

        xt = np.empty((D, TT), dtype=bf)
        xt[:, :C] = x_flat[gather_idx].T
        xt[:, C:] = x_flat[e * S0 : (e + 1) * S0].T

        cwv = np.ones(TT, dtype=np.float32)
        cwv[: len(idx)] = w_lists[e]
        cwv[len(idx) : C] = 0.0
        cwv = np.ascontiguousarray(cwv.reshape(TT // P, P).T)

        in_maps.append(
            {
                "xt": xt,
                "wg_e": np.ascontiguousarray(Wg[e].T).astype(bf),
                "wu_e": np.ascontiguousarray(Wu[e].T).astype(bf),
                "wd_e": np.ascontiguousarray(Wd[e].T).astype(bf),
                "wg_s": wsg,
                "wu_s": wsu,
                "wd_s": wsd,
                "cw": cwv,
            }
        )
    return x, in_maps, idx_pad, C, S0, N


def _combine(x_shape, results, idx_pad, C, S0, N):
    acc = np.zeros((N + 1, D), dtype=np.float32)
    for e in range(E):
        ye = results[e]["y"]
        acc[idx_pad[e]] += ye[:C]
        acc[e * S0 : (e + 1) * S0] += ye[C:]
    return acc[:N].reshape(x_shape)


def kernel(**inputs) -> np.ndarray:
    x, in_maps, idx_pad, C, S0, N = _prepare(inputs)
    nc = build_kernel(C, S0)
    res = run_bass_kernel_spmd(nc, in_maps, core_ids=list(range(8)))
    return _combine(x.shape, [res.results[e] for e in range(E)], idx_pad, C, S0, N)
